# revision 9
# baseline (speedup 1.0000x reference)
"""Trainium2 Bass kernel for a binarized BasicBlock:

    y = depthwise3x3(x, binarize(dw_weight)) + binarize(dw_bias)
    z = relu(binarize(pw_weight) @ y + binarize(pw_bias))

with x: [32, 256, 56, 56] fp32, stochastic BinaryConnect binarization using
jax.random.key(42) (bit-exact threefry, reproduced on host).

Strategy
--------
Data parallel over batch: 4 images per NeuronCore x 8 cores. All weights are
exactly +-1 after binarization, so the depthwise conv can be folded into the
1x1 "pointwise" GEMM:

    z[o, p] = sum_{tap k} (wp[o,c] * wd[c,k]) @ x[c, p + shift_k]

i.e. 9 GEMM passes with shifted views of x accumulating into the same PSUM
tile. The +-1 weight products are computed on host (exact in bf16). The
depthwise bias folds into a per-channel output bias:
bias_total[o] = sum_c wp[o,c]*bd[c] + bp[o] (small exact integers).

x is pre-padded (58x58) and cast to bf16 on host; PSUM accumulates in fp32;
ReLU+bias happens on the scalar engine during PSUM->SBUF eviction.

A fraction of the taps is optionally pre-combined on the vector engine
(scalar_tensor_tensor chains with per-partition +-1 scalars) to trade tensor
engine passes for vector engine work - tuned via N_PE_TAPS.
"""

import os

import numpy as np
import ml_dtypes

# ---------------------------------------------------------------------------
# problem constants (hardcoded; kernel.py must be self-contained)
# ---------------------------------------------------------------------------
B, C, H, W = 32, 256, 56, 56
O = 256
N_CORES = 8
NIMG = B // N_CORES          # images per core
HP, WP = H + 2, W + 2        # zero-padded spatial dims
S = HP * WP                  # padded image size per channel
P = 128                      # partitions
G = C // P                   # channel groups (2)
GO = O // P                  # output channel groups (2)
RB = 8                       # output rows per matmul tile
NT = H // RB                 # row tiles per image (7)
NFREE = RB * W               # matmul free size (448)

TAPS = [(ky, kx) for ky in range(3) for kx in range(3)]

_cache = {}


# ---------------------------------------------------------------------------
# host-side: reproduce the reference's stochastic binarization (threefry is
# bit-exact and platform independent)
# ---------------------------------------------------------------------------
def _binarize_all(dw_weight, dw_bias, pw_weight, pw_bias):
    import jax

    try:
        cpu = jax.devices("cpu")[0]
        ctx = jax.default_device(cpu)
    except Exception:  # pragma: no cover - cpu platform should always exist
        import contextlib

        ctx = contextlib.nullcontext()

    with ctx:
        rk = jax.random.key(42)
        k1, k2, k3, k4 = jax.random.split(rk, 4)

        def binarize(w, key):
            r = jax.random.uniform(key, w.shape, dtype=jax.numpy.float32)
            r = np.asarray(r, np.float32) * 2.0 - 1.0
            return np.sign(np.asarray(w, np.float32) - r).astype(np.float32)

        wd = binarize(dw_weight, k1)[:, 0]        # [C, 3, 3]
        bd = binarize(dw_bias, k2)                # [C]
        wp = binarize(pw_weight, k3)[:, :, 0, 0]  # [O, C]
        bp = binarize(pw_bias, k4)                # [O]
    return wd, bd, wp, bp


# ---------------------------------------------------------------------------
# device kernel
# ---------------------------------------------------------------------------
def _build_module(repeat=1):
    import concourse.mybir as mybir
    import concourse.tile as tile
    from concourse import bacc

    nc = bacc.Bacc("TRN2", target_bir_lowering=False, debug=False)

    dtb = mybir.dt.bfloat16
    dtf = mybir.dt.float32

    x_d = nc.dram_tensor("xp", [NIMG, C, HP, WP], dtb, kind="ExternalInput").ap()
    wk_d = nc.dram_tensor("wk", [G, P, 9, O], dtb, kind="ExternalInput").ap()
    bias_d = nc.dram_tensor("bias", [GO, P], dtf, kind="ExternalInput").ap()
    z_d = nc.dram_tensor("z", [NIMG, O, H, W], dtf, kind="ExternalOutput").ap()

    with tile.TileContext(nc) as tc:
        with (
            tc.tile_pool(name="const", bufs=1) as constp,
            tc.tile_pool(name="xbuf", bufs=1) as xpool,
            tc.tile_pool(name="zbuf", bufs=1) as zpool,
            tc.tile_pool(name="psum", bufs=6, space="PSUM") as psump,
        ):
            wk_sb = constp.tile([P, G, 9, O], dtb)
            nc.sync.dma_start(wk_sb[:], wk_d.rearrange("g c t o -> c g t o"))
            bias_dma = constp.tile([P, GO], dtf)
            nc.sync.dma_start(bias_dma[:], bias_d.rearrange("g c -> c g"))
            # copy on the scalar engine so the activations' bias dependency is
            # same-engine program order (the ACT ISA slot fits only one wait)
            bias_sb = constp.tile([P, GO], dtf)
            nc.scalar.copy(bias_sb[:], bias_dma[:])

            x_sb = xpool.tile([P, G, NIMG, HP, WP], dtb)
            for n in range(NIMG):
                for g in range(G):
                    nc.sync.dma_start(
                        x_sb[:, g, n], x_d[n, g * P : (g + 1) * P]
                    )

            # persistent output staging: single writer per region, so the
            # activations never pick up a WAR wait on an outbound DMA
            z_sb = zpool.tile([P, NIMG, GO, H, W], dtf)

            for _rep in range(repeat):
                _compute_body(nc, psump, wk_sb, bias_sb, x_sb, z_sb, z_d)
    nc.compile()
    return nc


def _compute_body(nc, psump, wk_sb, bias_sb, x_sb, z_sb, z_d):
    import concourse.mybir as mybir

    dtf = mybir.dt.float32
    for n in range(NIMG):
        for go in range(GO):
            for rb in range(NT):
                ps = psump.tile([P, RB, W], dtf)
                n_mms = len(TAPS) * G
                i = 0
                for t, (ky, kx) in enumerate(TAPS):
                    for g in range(G):
                        nc.tensor.matmul(
                            ps[:],
                            wk_sb[:, g, t, go * P : (go + 1) * P],
                            x_sb[
                                :, g, n,
                                rb * RB + ky : rb * RB + ky + RB,
                                kx : kx + W,
                            ],
                            start=(i == 0),
                            stop=(i == n_mms - 1),
                        )
                        i += 1
                nc.scalar.activation(
                    z_sb[:, n, go, rb * RB : (rb + 1) * RB, :],
                    ps[:],
                    mybir.ActivationFunctionType.Relu,
                    bias=bias_sb[:, go : go + 1],
                    scale=1.0,
                )
            nc.sync.dma_start(
                z_d[n, go * P : (go + 1) * P, :, :],
                z_sb[:, n, go],
            )


def _get_module(repeat=1):
    key = ("nc", repeat)
    if key not in _cache:
        _cache[key] = _build_module(repeat)
    return _cache[key]


# ---------------------------------------------------------------------------
# entry point
# ---------------------------------------------------------------------------
def kernel(x, dw_weight, dw_bias, pw_weight, pw_bias, _repeat=1):
    from concourse.bass_utils import run_bass_kernel_spmd

    x = np.asarray(x, np.float32)
    wd, bd, wp, bp = _binarize_all(dw_weight, dw_bias, pw_weight, pw_bias)

    # fold depthwise bias through the pointwise conv: exact small integers
    bias_total = (wp @ bd + bp).astype(np.float32)          # [O]
    bias_host = bias_total.reshape(GO, P)

    # folded per-tap weights: lhsT layout [g, c_local, tap, o] (+-1, exact bf16)
    wk_host = np.empty((G, P, 9, O), dtype=ml_dtypes.bfloat16)
    wd_flat = wd.reshape(C, 9)                              # [C, tap]
    for g in range(G):
        cs = slice(g * P, (g + 1) * P)
        # wp[o, c] * wd[c, t] -> [c_local, t, o]
        wk_host[g] = (wp.T[cs, None, :] * wd_flat[cs, :, None]).astype(
            ml_dtypes.bfloat16
        )

    # pre-padded bf16 input
    x_pad = np.zeros((B, C, HP, WP), dtype=ml_dtypes.bfloat16)
    x_pad[:, :, 1 : H + 1, 1 : W + 1] = x.astype(ml_dtypes.bfloat16)

    nc = _get_module(_repeat)

    in_maps = []
    for core in range(N_CORES):
        in_maps.append(
            {
                "xp": np.ascontiguousarray(x_pad[core * NIMG : (core + 1) * NIMG]),
                "wk": wk_host,
                "bias": bias_host,
            }
        )

    res = run_bass_kernel_spmd(
        nc,
        in_maps,
        core_ids=list(range(N_CORES)),
        trace=False,
    )
    _cache["last_result"] = res

    z = np.concatenate([r["z"] for r in res.results], axis=0)
    return z.astype(np.float32)
